# revision 1
# baseline (speedup 1.0000x reference)
"""Causal self-attention (RoPE + qk-RMS-norm) Trainium2 kernel.

Sharding: 8 cores = 2 batches x 4 head-groups (tensor-parallel over heads,
data-parallel over batch). Each core computes its head-group's attention and
a row-parallel partial of the output projection; the host sums the 4
per-group partials per batch (the all-reduce of row-parallel sharding).

Per-core layout: Q.T/K.T computed directly in [d, t] (no transposes),
V in [t, d]. Transposed flash attention: S.T = K @ Q.T so P.T feeds the
PV matmul directly; softmax has no max-subtraction (RMS-normed scores are
bounded by sqrt(D)); column sums via ones-matmul; 1/sum deferred to Y.T.
Matmuls run in float32r (full PE rate for N>=256). Tokens are processed in
two causal passes (halves of T) to fit SBUF.
"""

import functools

import numpy as np

B, T, C, H, D = 2, 2048, 1280, 10, 128
EPS = 1e-5
NHL = 3  # head slots per core (padded)
N_CORES = 8
NHALF = 2  # causal passes over T
# per-batch head groups (4th group padded with zero heads)
GROUPS = [[0, 1, 2], [3, 4, 5], [6, 7, 8], [9]]


def _emit(nc, tile, mybir, T, C, D, NHL, eps):
    F32 = mybir.dt.float32
    F32R = mybir.dt.float32r
    BF16 = mybir.dt.bfloat16
    I32 = mybir.dt.int32
    ActF = mybir.ActivationFunctionType
    Alu = mybir.AluOpType
    CCH = C // 128  # contraction chunks
    TBN = T // 128  # 128-token blocks
    T2 = T // NHALF  # tokens per pass
    TB2 = T2 // 128
    Q42 = T2 // 512  # q supertiles per pass
    HD = NHL * D
    couts = []
    off = 0
    while off < C:
        w = min(512, C - off)
        couts.append((off, w))
        off += w

    xt = nc.dram_tensor("xt", [C, T], BF16, kind="ExternalInput")
    wqt = nc.dram_tensor("wqt", [C, HD], BF16, kind="ExternalInput")
    wkt = nc.dram_tensor("wkt", [C, HD], BF16, kind="ExternalInput")
    wvt = nc.dram_tensor("wvt", [C, HD], BF16, kind="ExternalInput")
    wpt = nc.dram_tensor("wpt", [HD, C], BF16, kind="ExternalInput")
    cs = nc.dram_tensor("cs", [D, T], BF16, kind="ExternalInput")
    sc = nc.dram_tensor("sc", [D, T], BF16, kind="ExternalInput")
    # host-precomputed constants: [tri01 | ma | mb] (causal 128-block mask
    # and the two rope half-mix selector matrices)
    kconsts = nc.dram_tensor("kconsts", [128, 512], BF16, kind="ExternalInput")
    out = nc.dram_tensor("out", [T, C], BF16, kind="ExternalOutput")

    from contextlib import ExitStack

    with ExitStack() as ctx:
        ctx.enter_context(nc.allow_low_precision(reason="fp32r matmul operands"))
        tc = ctx.enter_context(tile.TileContext(nc))
        pool = lambda n, b, **kw: ctx.enter_context(tc.tile_pool(name=n, bufs=b, **kw))
        drp = pool("dr", 2, space="DRAM")
        per = pool("persist", 1)
        wvp = pool("wv", 1)
        wqkp = pool("wqk", 1)
        wptp = pool("wpt", 1)
        xtp = pool("xt", 2)
        qtp = pool("qt", 2)
        qsp = pool("qs", 1)
        ytp = pool("yt", 1)
        tmp = pool("tmp", 2)
        sqp = pool("sqp", 1)
        ptp = pool("ptp", 3)
        rows = pool("rows", 2)
        oev = pool("oev", 2)
        psmm = pool("psmm", 2, space="PSUM")
        psacc = pool("psacc", 2, space="PSUM")
        psrow = pool("psrow", 2, space="PSUM")

        # ---- constants ----
        ones_f = per.tile([128, 128], F32, tag="onf")
        nc.vector.memset(ones_f[:], 1.0)
        zeros_f = per.tile([128, 384], BF16, tag="zf")
        nc.vector.memset(zeros_f[:], 0.0)
        ones_col = per.tile([128, 1], BF16, tag="onc")
        nc.scalar.copy(ones_col[:], ones_f[:, 0:1])
        ones_row = per.tile([1, 128], BF16, tag="onr")
        nc.scalar.copy(ones_row[:], ones_f[0:1, :])
        ones_row32 = per.tile([1, 128], F32R, tag="onr32")
        nc.scalar.copy(ones_row32[:], ones_f[0:1, :])
        # constants tile: [tri01 | ma | mb] DMA'd from the host (keeps the
        # startup free of gpsimd affine_select work)
        kcon = per.tile([128, 512], BF16, tag="kcon")
        nc.sync.dma_start(kcon[:], kconsts[:, :])
        tri01 = kcon[:, 0:128]
        ma = kcon[:, 128:256]
        mb = kcon[:, 256:384]
        ident = kcon[:, 384:512]

        # PE warm-up: dummy accumulating matmuls during the initial DMA ramp
        warm = nc.dram_tensor("warm", [1, 512], F32, kind="ExternalOutput")
        wrhs = per.tile([128, 512], BF16, tag="wrhs")
        nc.vector.memset(wrhs[:], 1.0)
        wps = psrow.tile([1, 512], F32, tag="row", name="warmps")
        NWARM = 24
        for i in range(NWARM):
            nc.tensor.matmul(
                wps[:], ones_col[:], wrhs[:], start=(i == 0), stop=(i == NWARM - 1)
            )
        wsb = rows.tile([1, 512], F32, tag="rw", name="warmsb")
        nc.vector.tensor_copy(wsb[:], wps[:])
        nc.sync.dma_start(warm[:], wsb[:])

        wv = []
        wqr = []
        wkr = []
        # V for all heads/all tokens: [tk-part, tb, h, d]
        v_t = per.tile([128, TBN, NHL, D], BF16, tag="v")
        # K.T per head, all tokens
        ktr = [per.tile([128, T], BF16, tag=f"ktr{h}", name=f"ktr{h}")
               for h in range(NHL)]
        rk_cols = [per.tile([128, TBN], F32, tag=f"rkc{h}", name=f"rkc{h}")
                   for h in range(NHL)]

        # output-projection weights (resident); loaded late (first needed at
        # the pass-0 output projection ~150us in) so the transfers don't
        # compete with the startup x/wv/wq/wk loads for HBM bandwidth
        wp = {}

        def load_wp():
            for hh in range(NHL):
                for ci, (co, cw) in enumerate(couts):
                    t = wptp.tile([128, cw], BF16, tag=f"wp{hh}_{ci}")
                    nc.sync.dma_start(
                        t[:], wpt[hh * 128 : (hh + 1) * 128, co : co + cw]
                    )
                    wp[(hh, ci)] = t

        def emit_attention(hf, h, qtn, ytn, rwr):
            """Attention for head h over this pass's q supertiles.
            kb-outer (K/V stationary reuse); st/exp run one kb ahead of
            PV/colsum so the in-order PE queue never waits on ACT."""
            # deferred q-norm: broadcast the (long since DRAM-bounced) rsqrt
            # row over partitions and scale qtn; running this a full
            # iteration after the bounce keeps the PE from waiting on it
            for q4 in range(Q42):
                lsl = slice(q4 * 512, (q4 + 1) * 512)
                bq = psmm.tile([128, 512], F32, tag="mm", name="bq")
                nc.tensor.matmul(
                    bq[:], ones_row32[:], rwr[:, lsl], start=True, stop=True
                )
                nc.vector.tensor_mul(qtn[:, lsl], qtn[:, lsl], bq[:])
            gq4s = [hf * Q42 + q4 for q4 in range(Q42)]
            yts = [psacc.tile([128, 512], F32, tag="acc", name=f"yt{q4}")
                   for q4 in range(Q42)]
            # P column-sum accumulator (DVE bf16: 2x mode, light queue load)
            pacc = sqp.tile([128, Q42 * 512], BF16, tag="pacc", bufs=2)
            kbmax = 4 * (gq4s[-1] + 1)
            LA = 2  # st/exp run this many kb steps ahead of PV
            pts = {}  # kb -> pair pt tile awaiting PV
            for kb in range(kbmax + LA):
                if kb < kbmax:
                    active = [q4 for q4 in range(Q42) if kb <= 4 * gq4s[q4] + 3]
                    st = psmm.tile([128, Q42 * 512], F32, tag="mm", name="st")
                    for q4 in active:
                        lsl = slice(q4 * 512, (q4 + 1) * 512)
                        nc.tensor.matmul(
                            st[:, lsl],
                            ktr[h][:, kb * 128 : (kb + 1) * 128],
                            qtn[:, lsl],
                            start=True, stop=True,
                        )
                    pt = ptp.tile([128, Q42 * 512], BF16, tag="pt")
                    # one exp over the contiguous valid span of all active q4s
                    q0 = active[0]
                    j0 = kb - 4 * gq4s[q0]
                    lo = q0 * 512 + (j0 * 128 if j0 > 0 else 0)
                    hi = (active[-1] + 1) * 512
                    nc.scalar.activation(
                        pt[:, lo:hi], st[:, lo:hi], ActF.Exp,
                        scale=rk_cols[h][:, kb : kb + 1],
                    )
                    if 0 <= j0 <= 3:
                        dg = slice(q0 * 512 + j0 * 128, q0 * 512 + (j0 + 1) * 128)
                        nc.vector.tensor_mul(pt[:, dg], pt[:, dg], tri01)
                    # the strictly-above-diagonal prefix of the first active
                    # q4 is never written: narrow the pacc add (and the PV
                    # matmul below) to the valid span instead of zero-filling
                    for q4 in active:
                        a0 = q4 * 512 + (j0 * 128 if q4 == q0 and j0 > 0 else 0)
                        lsl = slice(a0, (q4 + 1) * 512)
                        if kb == 0:
                            nc.vector.tensor_copy(pacc[:, lsl], pt[:, lsl])
                        else:
                            nc.vector.tensor_add(
                                pacc[:, lsl], pacc[:, lsl], pt[:, lsl]
                            )
                    pts[kb] = pt
                if kb >= LA:
                    pkb = kb - LA
                    pt = pts.pop(pkb)
                    for q4 in range(Q42):
                        gq4 = gq4s[q4]
                        last_kb = 4 * gq4 + 3
                        if pkb > last_kb:
                            continue
                        jp = pkb - 4 * gq4
                        w0 = jp * 128 if jp > 0 else 0
                        nc.tensor.matmul(
                            yts[q4][:, w0:],
                            v_t[:, pkb, h, :],
                            pt[:, q4 * 512 + w0 : (q4 + 1) * 512],
                            start=(pkb == 0), stop=(pkb == last_kb),
                        )
            csrs = []
            for q4 in range(Q42):
                csum = psrow.tile([1, 512], F32, tag="row", name=f"cs{q4}")
                nc.tensor.matmul(
                    csum[:], ones_col[:], pacc[:, q4 * 512 : (q4 + 1) * 512],
                    start=True, stop=True,
                )
                csr = rows.tile([1, 512], BF16, tag="csr", name="csr", bufs=4)
                nc.vector.tensor_copy(csr[:], csum[:])
                csrs.append(csr)

            def normalize(h=h, ytn=ytn, yts=yts, csrs=csrs):
                for q4 in range(Q42):
                    lsl = slice(q4 * 512, (q4 + 1) * 512)
                    bc = psmm.tile([128, 512], F32, tag="mm", name="bc")
                    nc.tensor.matmul(
                        bc[:], ones_row[:], csrs[q4][:], start=True, stop=True
                    )
                    bcs = tmp.tile([128, 512], F32, tag="bcs", name="bcs")
                    nc.vector.reciprocal_approx_fast(bcs[:], bc[:])
                    nc.vector.tensor_mul(ytn[:, h, lsl], yts[q4][:], bcs[:])

            return normalize

        pending = None  # deferred attention emitter for the previous head
        pending_norm = None  # deferred normalize for the head before that

        def load_xc(hf_):
            # descriptor generation is ~600ns per issue and serial per engine:
            # alternate gpsimd/scalar, first token-halves of every chunk
            # before second halves so the leading vproj/qkproj tiles can
            # start while the rest streams in
            toff_ = hf_ * T2
            xcl = []
            for c in range(CCH):
                t = xtp.tile([128, T2], BF16, tag=f"x{c}", name=f"x{c}")
                xcl.append(t)
            for half in range(2):
                tsl = slice(half * (T2 // 2), (half + 1) * (T2 // 2))
                for c in range(CCH):
                    (nc.gpsimd if c % 2 == 0 else nc.scalar).dma_start(
                        xcl[c][:, tsl],
                        xt[c * 128 : (c + 1) * 128,
                           toff_ + half * (T2 // 2) :
                           toff_ + (half + 1) * (T2 // 2)],
                    )
            return xcl

        xc_next = None
        for hf in range(NHALF):
            toff = hf * T2
            # ---- per-pass cos/sin (stacked) ----
            cs_t = qtp.tile([D, T2], BF16, tag="cs", bufs=1)
            sc_t = qtp.tile([D, T2], BF16, tag="sc", bufs=1)
            nc.sync.dma_start(cs_t[:], cs[:, toff : toff + T2])
            nc.sync.dma_start(sc_t[:], sc[:, toff : toff + T2])
            # ---- x.T chunks: loaded here for pass 0, prefetched mid-pass-0
            # (double-buffered) for pass 1 ----
            if xc_next is not None:
                xc = xc_next
                xc_next = None
            else:
                # pass-0 startup: issue in consumption-priority order so the
                # first vproj/qkproj tiles aren't stuck behind late-needed
                # bytes -- x first halves, wv, wq, x second halves, wk --
                # alternating the two DMA-capable compute engines throughout
                xc = []
                for c in range(CCH):
                    t = xtp.tile([128, T2], BF16, tag=f"x{c}", name=f"x{c}")
                    xc.append(t)

                def xc_issue(half):
                    tsl = slice(half * (T2 // 2), (half + 1) * (T2 // 2))
                    for c in range(CCH):
                        (nc.gpsimd if c % 2 == 0 else nc.scalar).dma_start(
                            xc[c][:, tsl],
                            xt[c * 128 : (c + 1) * 128,
                               toff + half * (T2 // 2) :
                               toff + (half + 1) * (T2 // 2)],
                        )

                xc_issue(0)
                for c in range(CCH):
                    t = wvp.tile([128, HD], BF16, tag=f"wv{c}")
                    (nc.scalar if c % 2 == 0 else nc.gpsimd).dma_start(
                        t[:], wvt[c * 128 : (c + 1) * 128, :]
                    )
                    wv.append(t)
                for c in range(CCH):
                    tq = wqkp.tile([128, HD], BF16, tag=f"wq{c}")
                    (nc.gpsimd if c % 2 == 0 else nc.scalar).dma_start(
                        tq[:], wqt[c * 128 : (c + 1) * 128, :]
                    )
                    wqr.append(tq)
                xc_issue(1)
                for c in range(CCH):
                    tk = wqkp.tile([128, HD], BF16, tag=f"wk{c}")
                    (nc.gpsimd if c % 2 == 0 else nc.scalar).dma_start(
                        tk[:], wkt[c * 128 : (c + 1) * 128, :]
                    )
                    wkr.append(tk)

            # ---- V projection for this pass, all heads batched ----
            for tb in range(TB2):
                gtb = hf * TB2 + tb
                vp = psmm.tile([128, HD], F32, tag="mm", name="vp")
                for c in range(CCH):
                    nc.tensor.matmul(
                        vp[:],
                        xc[c][:, tb * 128 : (tb + 1) * 128],
                        wv[c][:],
                        start=(c == 0), stop=(c == CCH - 1),
                    )
                nc.vector.tensor_copy(v_t[:, gtb, :, :], vp[:])

            # Y.T for this pass (all heads)
            ytn = ytp.tile([128, NHL, T2], BF16, tag="ytn")

            for h in range(NHL):
                if hf == 0 and h == 1:
                    load_wp()
                if hf + 1 < NHALF and h == NHL - 1:
                    xc_next = load_xc(hf + 1)
                # ---- Q/K projections into PSUM, evicted early to SBUF ----
                hds = slice(h * D, (h + 1) * D)
                qsb = {}
                for isq, wt in enumerate((wqr, wkr)):
                    qps = psmm.tile([128, Q42 * 512], F32, tag="mm", name="qps")
                    for c in range(CCH):
                        for q4 in range(Q42):
                            nc.tensor.matmul(
                                qps[:, q4 * 512 : (q4 + 1) * 512],
                                wt[c][:, hds],
                                xc[c][:, q4 * 512 : (q4 + 1) * 512],
                                start=(c == 0), stop=(c == CCH - 1),
                            )
                    for q4 in range(Q42):
                        sb = qsp.tile([128, 512], BF16, tag=f"qs{isq}{q4}")
                        # q evictions on DVE (feed the critical rope->norm
                        # chain), k evictions on scalar to split queue load
                        if isq == 0:
                            nc.vector.tensor_copy(
                                sb[:], qps[:, q4 * 512 : (q4 + 1) * 512]
                            )
                        else:
                            nc.scalar.copy(
                                sb[:], qps[:, q4 * 512 : (q4 + 1) * 512]
                            )
                        qsb[(isq, q4)] = sb

                qtn = qtp.tile([128, T2], BF16, tag="qtn")

                # ---- rope + norm ----
                # Sum-of-squares lands as per-128-block COLUMNS (tiny N=1
                # matmuls), rsqrt is a quake-style bit-trick + 2 Newton steps
                # on DVE -- no scalar Sqrt/Ln, so the Exp act table is never
                # swapped out.
                nrm = psrow.tile([128, 2 * Q42 * 4], F32, tag="row", name="nrm")
                rope_io = [(qtn, 0), (ktr[h], toff)]
                # phase A: all 8 rope input products on gpsimd up front; they
                # compute while the PE is still on this head's projections /
                # the previous head's attention, so the rope matmuls below
                # never crawl at gpsimd pace
                tprod = {}
                for isq in range(2):
                    # q products on DVE (needed first, bf16 2x rate); k
                    # products on gpsimd (needed later, hidden by attention)
                    eng = nc.vector if isq == 0 else nc.gpsimd
                    for q4 in range(Q42):
                        qp = qsb[(isq, q4)]
                        lsl4 = slice(q4 * 512, (q4 + 1) * 512)
                        t1 = tmp.tile([128, 512], BF16, tag=f"t1{isq}{q4}")
                        t2 = tmp.tile([128, 512], BF16, tag=f"t2{isq}{q4}")
                        eng.tensor_mul(t1[:], qp[:], cs_t[:, lsl4])
                        eng.tensor_mul(t2[:], qp[:], sc_t[:, lsl4])
                        tprod[(isq, q4)] = (t1, t2)
                for isq, (dst, doff) in enumerate(rope_io):
                    # phase B: rope matmuls + evictions
                    for q4 in range(Q42):
                        dsl = slice(doff + q4 * 512, doff + (q4 + 1) * 512)
                        t1, t2 = tprod[(isq, q4)]
                        rp = psmm.tile([128, 512], F32, tag="mm", name="rp")
                        nc.tensor.matmul(rp[:], ma, t1[:], start=True, stop=False)
                        nc.tensor.matmul(rp[:], mb, t2[:], start=False, stop=True)
                        nc.scalar.copy(dst[:, dsl], rp[:])
                    # phase C: squares + per-block column reduces
                    for q4 in range(Q42):
                        dsl = slice(doff + q4 * 512, doff + (q4 + 1) * 512)
                        sq = sqp.tile([128, 512], BF16, tag="sq")
                        nc.vector.tensor_mul(sq[:], dst[:, dsl], dst[:, dsl])
                        for b in range(4):
                            co = isq * 8 + q4 * 4 + b
                            nc.tensor.matmul(
                                nrm[:, co : co + 1],
                                sq[:, b * 128 : (b + 1) * 128], ones_col[:],
                                start=True, stop=True,
                            )
                # rsqrt chain on [128, 16]: cols 0:8 = q (no eps; pad heads
                # get nonzero Wq host-side), cols 8:16 = k (ssk/D + eps)
                nsb = rows.tile([128, 16], F32, tag="nsb")
                nc.vector.tensor_copy(nsb[:, 0:8], nrm[:, 0:8])
                nc.vector.tensor_scalar(
                    nsb[:, 8:16], nrm[:, 8:16], 1.0 / D, float(eps),
                    op0=Alu.mult, op1=Alu.add,
                )
                ysb = rows.tile([128, 16], F32, tag="ysb")
                nsi = nsb[:].bitcast(I32)
                ysi = ysb[:].bitcast(I32)
                nc.vector.tensor_scalar(
                    ysi, nsi, 1, None, op0=Alu.logical_shift_right
                )
                nc.vector.tensor_scalar(
                    ysi, ysi, 0x5F3759DF, -1, op0=Alu.subtract, op1=Alu.mult
                )
                ntmp = rows.tile([128, 16], F32, tag="ntmp")
                for _ in range(2):
                    nc.vector.tensor_mul(ntmp[:], ysb[:], ysb[:])
                    nc.vector.tensor_mul(ntmp[:], ntmp[:], nsb[:])
                    nc.vector.tensor_scalar(
                        ntmp[:], ntmp[:], -0.5, 1.5, op0=Alu.mult, op1=Alu.add
                    )
                    nc.vector.tensor_mul(ysb[:], ysb[:], ntmp[:])
                # k: rsqrt columns drop straight into rk_cols (no transpose)
                nc.vector.tensor_copy(
                    rk_cols[h][:, hf * TB2 : (hf + 1) * TB2], ysb[:, 8:16]
                )
                # q: bounce [128, 8] cols -> [1, T2] row through DRAM, then
                # broadcast over partitions via ones-outer matmul per q4
                rqc = rows.tile([128, 8], F32R, tag="rqc")
                nc.vector.tensor_copy(rqc[:], ysb[:, 0:8])
                rkd = drp.tile([1, T2], F32R, tag="rkd")
                nc.sync.dma_start(
                    rkd[0:1, :].rearrange("a (j p) -> a p j", p=128), rqc[:]
                )
                rwr = rows.tile([1, T2], F32R, tag="rwr", bufs=2)
                nc.sync.dma_start(rwr[:], rkd[:])

                # ---- normalize for head h-2 (frees its PSUM accumulators),
                # then attention for head h-1; this head's rope/norm chain
                # above overlaps the attention so the PE never waits on it ----
                if pending_norm is not None:
                    pending_norm()
                    pending_norm = None
                if pending is not None:
                    pending_norm = pending()
                    pending = None

                pending = (lambda hf=hf, h=h, qtn=qtn, ytn=ytn, rwr=rwr:
                           emit_attention(hf, h, qtn, ytn, rwr))

            # ---- last head's attention + normalizes, then output proj ----
            if pending_norm is not None:
                pending_norm()
                pending_norm = None
            if pending is not None:
                norm_last = pending()
                pending = None
                norm_last()
            for tb in range(TB2):
                for ci, (co, cw) in enumerate(couts):
                    op = psacc.tile([128, cw], F32, tag="acc", name="op")
                    for hh in range(NHL):
                        nc.tensor.matmul(
                            op[:],
                            ytn[:, hh, tb * 128 : (tb + 1) * 128],
                            wp[(hh, ci)][:],
                            start=(hh == 0), stop=(hh == NHL - 1),
                        )
                    ot = oev.tile([128, cw], BF16, tag="otb")
                    if (tb * len(couts) + ci) % 2 == 0:
                        nc.vector.tensor_copy(ot[:], op[:])
                    else:
                        nc.scalar.copy(ot[:], op[:])
                    nc.sync.dma_start(
                        out[toff + tb * 128 : toff + (tb + 1) * 128, co : co + cw],
                        ot[:],
                    )
    return nc


@functools.lru_cache(maxsize=4)
def _build(T_=T, C_=C, D_=D, NHL_=NHL, eps=EPS):
    import concourse.bacc as bacc
    import concourse.tile as tile
    from concourse import mybir

    nc = bacc.Bacc("TRN2", target_bir_lowering=False)
    _emit(nc, tile, mybir, T_, C_, D_, NHL_, eps)
    nc.compile()
    return nc


def _shard(x, cos, sin, Wq, Wk, Wv, Wproj):
    """Build the 8 per-core input maps."""
    import ml_dtypes

    BF = ml_dtypes.bfloat16
    HD = NHL * D
    cosT = np.ascontiguousarray(cos[0, 0].T.astype(np.float32))  # [64, T]
    sinT = np.ascontiguousarray(sin[0, 0].T.astype(np.float32))
    cs = np.concatenate([cosT, sinT], axis=0).astype(BF)  # [128, T]
    sc = np.concatenate([sinT, cosT], axis=0).astype(BF)

    # [tri01 | ma | mb] constants (see _emit)
    kc = np.zeros((128, 512), np.float32)
    kc[:, 0:128] = np.triu(np.ones((128, 128), np.float32))
    for j in range(64):
        kc[j, 128 + j] = 1.0
        kc[64 + j, 128 + j] = 1.0
        kc[j, 256 + 64 + j] = -1.0
        kc[64 + j, 256 + 64 + j] = 1.0
    kc[:, 384:512] = np.eye(128, dtype=np.float32)
    kc = kc.astype(BF)

    def head_rows(W, heads, pad=0.0):
        rows = np.full((HD, C), pad, np.float32)
        for i, h in enumerate(heads):
            rows[i * D : (i + 1) * D] = W[h * D : (h + 1) * D]
        return rows

    in_maps = []
    for b in range(B):
        xtb = np.ascontiguousarray(x[b].T).astype(BF)  # [C, T]
        for heads in GROUPS:
            wq = np.ascontiguousarray(head_rows(Wq, heads, pad=0.01).T).astype(BF)
            wk = np.ascontiguousarray(head_rows(Wk, heads).T).astype(BF)
            wv = np.ascontiguousarray(head_rows(Wv, heads).T).astype(BF)
            # Wproj columns for these heads, transposed: [HD, C]
            wp = np.zeros((HD, C), np.float32)
            for i, h in enumerate(heads):
                wp[i * D : (i + 1) * D] = Wproj[:, h * D : (h + 1) * D].T
            in_maps.append(
                {"xt": xtb, "wqt": wq, "wkt": wk, "wvt": wv,
                 "wpt": wp.astype(BF), "cs": cs, "sc": sc, "kconsts": kc}
            )
    return in_maps


def _gather(results):
    y = np.zeros((B, T, C), np.float32)
    for b in range(B):
        for g in range(len(GROUPS)):
            y[b] += results[b * len(GROUPS) + g]["out"].astype(np.float32)
    return y


def _run(in_maps, trace=False):
    from concourse.bass_utils import run_bass_kernel_spmd

    nc = _build()
    return run_bass_kernel_spmd(
        nc, in_maps, core_ids=list(range(N_CORES)), trace=trace
    )


def kernel(x, cos, sin, Wq, Wk, Wv, Wproj):
    ins = _shard(
        np.asarray(x), np.asarray(cos), np.asarray(sin),
        np.asarray(Wq), np.asarray(Wk), np.asarray(Wv), np.asarray(Wproj),
    )
    res = _run(ins, trace=False)
    return _gather(res.results)


def run_traced(x, cos, sin, Wq, Wk, Wv, Wproj):
    ins = _shard(
        np.asarray(x), np.asarray(cos), np.asarray(sin),
        np.asarray(Wq), np.asarray(Wk), np.asarray(Wv), np.asarray(Wproj),
    )
    res = _run(ins, trace=True)
    return _gather(res.results), res

